# revision 44
# baseline (speedup 1.0000x reference)
"""Trainium2 Bass kernel for nn_CrossAttention (FFT-query cross attention).

Math:
  out = softmax((Re(FFT(query, axis=1)) @ Wq^T + bq) @ (key @ Wk^T + bk)^T / sqrt(D)) @ key

Identities used:
  * Re(FFT(x))[j] = sum_n x[n] cos(2*pi*j*n/N): a matmul with a cosine matrix.
  * Row mirror: out[b, j] == out[b, N-j]; device computes j = 0..1023, host
    computes the single row j=1024 directly and mirrors 1025..2047.
  * Column fold (twice):
      fold1: y[n] = x[n] + x[N-n]           (2048 -> 1025 terms)
      fold2: even j contract yE[n] = y[n]+y[1024-n] vs cos table [513 x 512];
             odd  j contract yO[n] = y[n]-y[1024-n] vs cos table [512 x 512].
    Total cosine table is half of fold1's, and the q-projection matmul halves.
  * bk drops out of softmax (constant per row); bq/16 is added via the qsT
    PSUM drain (per-partition scalar add), so no bias rows in any matmul.
  * 1/sqrt(D) is folded into the cosine table and bias.
  * Wk is folded into the query side: S = (qs @ Wk) @ key^T, so the key
    projection matmul over 2048 rows is replaced by a 256x256 one (qk = qs@Wk).
  * Softmax rowsum comes free out of the P@V matmul via a ones-column
    appended to the value matrix (no ACT accum, no separate reduction).

Per-core layout (core b handles batch b; 8 cores, 8 batches):
  A:  zE/zO[n, d] = y? @ Wq^T           lhsT = yt (host),    rhs = Wq^T (host)
  B:  qsT[d, j]   = z^T @ (C/16) + bq/16  (even|odd j halves; bias in drain)
  C': qkT[d, j]   = Wk^T-contraction of qsT   lhsT = Wk (host), rhs = qsT
  D:  S[j, nk]    = qk @ key^T          lhsT = qkT,          rhs = key^T (host)
      (per j-tile: 4 chunks of 512 keys; chunk max on DVE/Pool as it lands,
       exp per chunk on ACT with global row negmax bias; P in bf16)
  T:  P^T tiles via PE transpose (bf16), interleaved with
  E:  o[j, d]     = P @ [key | 1]       lhsT = P^T chunk,    rhs = key bf16
      col 256 of the accumulators is the softmax row sum; final scale by its
      reciprocal on DVE/Pool.

Scheduling notes:
  * f32r everywhere on the scores side (1 cycle/row when free size >= 256).
  * Per-jt issue order: D(jt) chunks -> softmax(jt) -> T/E(jt-1) interleaved,
    so the PE never waits on the ACT exp chain (chunk-granular pipelining).
  * PSUM: score chunks tag x4 banks, transpose tag x2, output tag x2 = 8.
"""

import numpy as np
import ml_dtypes

import concourse.bass as bass
import concourse.tile as tile
from concourse import bacc, mybir
from concourse.bass_utils import run_bass_kernel_spmd

B = 8
NSEQ = 2048          # query/key sequence length
D = 256              # feature dim
NJ = 1024            # device-computed query rows (512 even + 512 odd classes)
NE = 513             # even-class contraction length
NO = 512             # odd-class contraction length
NKT = NSEQ // 128    # 16 key tiles
SCALE = 1.0 / 16.0   # 1/sqrt(D)

f32 = mybir.dt.float32
f32r = mybir.dt.float32r
bf16 = mybir.dt.bfloat16

_compiled = {}


def _build_module():
    nc = bacc.Bacc("TRN2", target_bir_lowering=False, debug=False, num_devices=B)

    dram = {}
    def din(name, shape, dt=f32):
        dram[name] = nc.dram_tensor(name, list(shape), dt, kind="ExternalInput").ap()
    def dout(name, shape):
        dram[name] = nc.dram_tensor(name, list(shape), f32, kind="ExternalOutput").ap()

    din("yt", (D, 1025))          # [yE^T | yO^T] folded query, transposed
    din("wqt", (D, D))            # Wq^T / 16
    din("bqs", (D, 1))            # bq / 16
    din("wk", (D, D))             # Wk natural
    din("keyt", (D, NSEQ))        # key^T
    din("ident", (128, 128), bf16)
    din("ident32", (128, 128))
    dout("ob", (NJ, D))

    with tile.TileContext(nc) as tc:
        _emit(nc, tc, dram)
    nc.compile()
    return nc


def _emit(nc, tc, dram):
    from contextlib import ExitStack

    X = mybir.AxisListType.X
    EXP = mybir.ActivationFunctionType.Exp
    SIN = mybir.ActivationFunctionType.Sin
    ADD = mybir.AluOpType.add
    MULT = mybir.AluOpType.mult
    BAND = mybir.AluOpType.bitwise_and
    i32 = mybir.dt.int32
    PI = float(np.pi)

    with ExitStack() as ctx:
        const = ctx.enter_context(tc.tile_pool(name="const", bufs=1))
        zpool = ctx.enter_context(tc.tile_pool(name="z", bufs=1))
        qpool = ctx.enter_context(tc.tile_pool(name="q", bufs=1))
        gen = ctx.enter_context(tc.tile_pool(name="gen", bufs=1))

        # ---- constant loads, in phase-consumption order ----
        yt = [const.tile([128, 1025], f32r, tag=f"yt{i}", name=f"yt{i}") for i in range(2)]
        wqt = [const.tile([128, D], f32r, tag=f"wqt{i}", name=f"wqt{i}") for i in range(2)]
        bqs = [const.tile([128, 1], f32, tag=f"bqs{i}", name=f"bqs{i}") for i in range(2)]
        for i in range(2):
            nc.sync.dma_start(yt[i][:], dram["yt"][i * 128:(i + 1) * 128, :].bitcast(f32r))
            nc.sync.dma_start(wqt[i][:], dram["wqt"][i * 128:(i + 1) * 128, :].bitcast(f32r))
            nc.sync.dma_start(bqs[i][:], dram["bqs"][i * 128:(i + 1) * 128, :])

        # ---- cosine tables generated on device (no DMA, no deps):
        #   ctE[n, j'] = cos(pi*n*j'/512)  = sin(pi*v/512 - pi),
        #       v = (n*j' + 768) mod 1024
        #   ctO[n, j'] = cos(pi*n*(2j'+1)/1024) = sin(pi*v/1024 - pi),
        #       v = (n*(2j'+1) + 1536) mod 2048
        # All intermediates are integers < 2^24, exact in f32; Pool does the
        # arithmetic (idle in the prologue), ACT evaluates Sin on [-pi, pi).
        MOD = mybir.AluOpType.mod
        negpi = gen.tile([128, 1], f32, tag="negpi", name="negpi")
        nc.gpsimd.memset(negpi[:], -PI)
        mask_e = gen.tile([128, 1], i32, tag="mask_e", name="mask_e")
        nc.gpsimd.memset(mask_e[:], 1023)
        mask_o = gen.tile([128, 1], i32, tag="mask_o", name="mask_o")
        nc.gpsimd.memset(mask_o[:], 2047)
        jotf = gen.tile([128, 512], f32, tag="jotf", name="jotf")
        nc.gpsimd.iota(jotf[:], [[1, 512]], channel_multiplier=0,
                       allow_small_or_imprecise_dtypes=True)
        cte, cto = [], []
        for t in range(5):
            r = 128 if t < 4 else 1
            ncol = gen.tile([r, 1], f32, tag=f"ncol{t}", name=f"ncol{t}")
            nc.gpsimd.iota(ncol[:], [[0, 1]], base=t * 128, channel_multiplier=1,
                           allow_small_or_imprecise_dtypes=True)
            prod = gen.tile([r, 512], f32, tag=f"prod{t}", name=f"prod{t}")
            nc.vector.tensor_scalar(out=prod[:], in0=jotf[:r, :], scalar1=ncol[:],
                                    scalar2=768.0, op0=MULT, op1=ADD)
            pi_ = gen.tile([r, 512], i32, tag=f"pi{t}", name=f"pi{t}")
            nc.vector.tensor_copy(pi_[:], prod[:])
            ve = gen.tile([r, 512], i32, tag=f"ve{t}", name=f"ve{t}")
            nc.vector.tensor_scalar(out=ve[:], in0=pi_[:], scalar1=mask_e[:r, :],
                                    scalar2=None, op0=BAND)
            te = const.tile([r, 512], f32r, tag=f"cte{t}", name=f"cte{t}")
            nc.scalar.activation(out=te[:], in_=ve[:], func=SIN,
                                 scale=PI / 512.0, bias=negpi[:r, :])
            cte.append(te)
            if t < 4:
                prodo = gen.tile([128, 512], f32, tag=f"prodo{t}", name=f"prodo{t}")
                nc.vector.tensor_scalar(out=prodo[:], in0=prod[:], scalar1=2.0,
                                        scalar2=ncol[:], op0=MULT, op1=ADD)
                po_ = gen.tile([128, 512], i32, tag=f"po{t}", name=f"po{t}")
                nc.vector.tensor_copy(po_[:], prodo[:])
                vo = gen.tile([128, 512], i32, tag=f"vo{t}", name=f"vo{t}")
                nc.vector.tensor_scalar(out=vo[:], in0=po_[:], scalar1=mask_o[:],
                                        scalar2=None, op0=BAND)
                to = const.tile([128, 512], f32r, tag=f"cto{t}", name=f"cto{t}")
                nc.scalar.activation(out=to[:], in_=vo[:], func=SIN,
                                     scale=PI / 1024.0, bias=negpi[:])
                cto.append(to)

        wk = [const.tile([128, D], f32r, tag=f"wk{i}", name=f"wk{i}") for i in range(2)]
        for i in range(2):
            nc.sync.dma_start(wk[i][:], dram["wk"][i * 128:(i + 1) * 128, :].bitcast(f32r))
        keyt = [[None] * 4 for _ in range(2)]
        for kc in range(4):
            for dt in range(2):
                t = const.tile([128, 512], f32r, tag=f"keyt{dt}_{kc}", name=f"keyt{dt}_{kc}")
                nc.sync.dma_start(
                    t[:], dram["keyt"][dt * 128:(dt + 1) * 128,
                                       kc * 512:(kc + 1) * 512].bitcast(f32r))
                keyt[dt][kc] = t
        keynf = [const.tile([128, 1028], bf16, tag=f"keynf{g}", name=f"keynf{g}")
                 for g in range(4)]
        for g in range(4):
            for q in range(4):
                nc.gpsimd.memset(keynf[g][:, q * 257 + 256:q * 257 + 257], 1.0)
        id_b = const.tile([128, 128], bf16, tag="ident", name="ident")
        nc.sync.dma_start(id_b[:], dram["ident"][:])
        id_f = const.tile([128, 128], f32r, tag="ident32", name="ident32")
        nc.sync.dma_start(id_f[:], dram["ident32"][:].bitcast(f32r))

        # prewarm the ACT exp table so the first real exp doesn't eat the
        # 1.3us ACT_TABLE_LOAD on the critical path; reading a Sin output
        # forces this after the (reordering) scheduler runs all the Sins
        warm2 = const.tile([1, 1], f32, tag="warm2", name="warm2")
        nc.scalar.activation(out=warm2[:], in_=cto[3][:1, :1],
                             func=EXP, scale=0.0)

        # ---- phases A+B fused: B accumulation step kt only needs z tile
        # kt, so A chains feed B steps with a skew of one.  z tiles: 0..4
        # even-class rows (128,128,128,128,1), 5..8 odd-class.
        IDENT = mybir.ActivationFunctionType.Identity
        zrows = [128, 128, 128, 128, 1, 128, 128, 128, 128]
        zcol0 = [0, 128, 256, 384, 512, 513, 641, 769, 897]
        zbuf = []
        for i in range(9):
            zbuf.append(zpool.tile([zrows[i], D], f32r, tag=f"z{i}", name=f"z{i}"))
        qsT = [qpool.tile([128, NJ], f32r, tag=f"qsT{i}", name=f"qsT{i}") for i in range(2)]
        qkT = [qpool.tile([128, NJ], f32r, tag=f"qkT{i}", name=f"qkT{i}") for i in range(2)]

        def a_chain(psA, nt):
            m = zrows[nt]
            pt = psA.tile([128, D], f32, tag="psA", name="psA")
            for kd in range(2):
                nc.tensor.matmul(
                    pt[:m, :], yt[kd][:, zcol0[nt]:zcol0[nt] + m],
                    wqt[kd][:], start=(kd == 0), stop=(kd == 1))
            nc.scalar.copy(zbuf[nt][:m, :], pt[:m, :])

        psB_ctx = tc.tile_pool(name="psB", bufs=1, space="PSUM")
        psB = psB_ctx.__enter__()
        pb = {}
        for dt in range(2):
            for h in range(2):  # 0 = even, 1 = odd
                pb[(dt, h)] = psB.tile([128, 512], f32, tag=f"psB{dt}{h}", name="psB")
        with tc.tile_pool(name="psA", bufs=4, space="PSUM") as psA:
            for kt in range(6):
                if kt < 5:
                    a_chain(psA, kt)
                    if kt < 4:
                        a_chain(psA, 5 + kt)
                if kt >= 1:
                    bt = kt - 1
                    kr = zrows[bt]
                    for dt in range(2):
                        nc.tensor.matmul(
                            pb[(dt, 0)][:], zbuf[bt][:kr, dt * 128:(dt + 1) * 128],
                            cte[bt][:kr, :], start=(bt == 0), stop=(bt == 4))
                    if bt < 4:
                        for dt in range(2):
                            nc.tensor.matmul(
                                pb[(dt, 1)][:], zbuf[5 + bt][:, dt * 128:(dt + 1) * 128],
                                cto[bt][:], start=(bt == 0), stop=(bt == 3))
        # B drains (+bias) on ACT; C' follows per half as its inputs land
        if True:
            with tc.tile_pool(name="psC", bufs=1, space="PSUM") as psC:
                pc = {}
                for dt in range(2):
                    for h in range(2):
                        pc[(dt, h)] = psC.tile([128, 512], f32, tag=f"psC{dt}{h}", name="psC")
                for h in range(2):
                    for dt in range(2):
                        nc.scalar.activation(
                            out=qsT[dt][:, h * 512:(h + 1) * 512], in_=pb[(dt, h)][:],
                            func=IDENT, bias=bqs[dt][:], scale=1.0)
                    for kd in range(2):
                        for dt in range(2):
                            nc.tensor.matmul(
                                pc[(dt, h)][:], wk[kd][:, dt * 128:(dt + 1) * 128],
                                qsT[kd][:, h * 512:(h + 1) * 512],
                                start=(kd == 0), stop=(kd == 1))
                    for dt in range(2):
                        nc.scalar.copy(qkT[dt][:, h * 512:(h + 1) * 512], pc[(dt, h)][:])
            psB_ctx.__exit__(None, None, None)

        # ---- phase D/T/E: attention over 8 query tiles, chunk-pipelined ----
        with ExitStack() as jctx:
            ps = jctx.enter_context(tc.tile_pool(name="ps", bufs=1, space="PSUM"))
            work = jctx.enter_context(tc.tile_pool(name="work", bufs=1))
            stats = jctx.enter_context(tc.tile_pool(name="stats", bufs=1))

            def d_chunk(jt, kc, scs, ms):
                jsl = slice(jt * 128, (jt + 1) * 128)
                s_c = ps.tile([128, 512], f32, tag="s", bufs=5, name="s")
                for dt in range(2):
                    nc.tensor.matmul(
                        s_c[:], qkT[dt][:, jsl], keyt[dt][kc],
                        start=(dt == 0), stop=(dt == 1))
                m = stats.tile([128, 1], f32, tag=f"m{kc}", bufs=2, name=f"m{kc}")
                nc.vector.reduce_max(out=m[:], in_=s_c[:], axis=X, negate=True)
                scs.append(s_c)
                ms.append(m)

            def softmax_head(jt, scs, ms):
                nm01 = stats.tile([128, 1], f32, tag="nm01", bufs=2, name="nm01")
                nc.gpsimd.tensor_scalar_min(nm01[:], ms[0][:], ms[1][:])
                nm23 = stats.tile([128, 1], f32, tag="nm23", bufs=2, name="nm23")
                nc.gpsimd.tensor_scalar_min(nm23[:], ms[2][:], ms[3][:])
                nmx = stats.tile([128, 1], f32, tag="nmx", bufs=2, name="nmx")
                nc.gpsimd.tensor_scalar_min(nmx[:], nm01[:], nm23[:])
                p_t = work.tile([128, NSEQ], bf16, tag="p", bufs=2, name="p")
                state[jt] = (p_t, scs, nmx)

            def exps(jt):
                # issued late so DVE/scalar queues ahead of them (pt drains,
                # recip) aren't blocked behind the exp chain
                p_t, scs, nmx = state[jt]
                for kc in range(4):
                    nc.scalar.activation(
                        out=p_t[:, kc * 512:(kc + 1) * 512], in_=scs[kc][:],
                        func=EXP, bias=nmx[:], scale=1.0)

            def t_half(p_t, hg, pt_sbs):
                pt_ps = ps.tile([128, 1024], bf16, tag="ptps", bufs=2, name="ptps")
                for q in range(8):
                    kt = hg * 8 + q
                    nc.tensor.matmul(pt_ps[:, q * 128:(q + 1) * 128],
                                     p_t[:, kt * 128:(kt + 1) * 128],
                                     id_b[:], is_transpose=True,
                                     start=True, stop=True)
                pt_sb = work.tile([128, 1024], bf16, tag="pt", bufs=2, name="pt")
                nc.vector.tensor_copy(pt_sb[:], pt_ps[:])
                pt_sbs[hg] = pt_sb

            state = {}
            # ---- value tiles keynf = key bf16, transposed out of keyt on
            # the PE, interleaved with D(0) as keyt chunks arrive ----
            with tc.tile_pool(name="ptk", bufs=1, space="PSUM") as ptk:
                scs0, ms0 = [], []
                for kc in range(4):
                    for half in range(2):
                        kb = ptk.tile([128, 512], f32r, tag="ptk", name="ptk")
                        for q2 in range(2):
                            kt = kc * 4 + half * 2 + q2
                            for dt in range(2):
                                nc.tensor.matmul(
                                    kb[:, (q2 * 2 + dt) * 128:(q2 * 2 + dt + 1) * 128],
                                    keyt[dt][kc][:, (half * 2 + q2) * 128:(half * 2 + q2 + 1) * 128],
                                    id_f[:], is_transpose=True, start=True, stop=True)
                        for q2 in range(2):
                            q = half * 2 + q2
                            nc.vector.tensor_copy(
                                keynf[kc][:, q * 257:q * 257 + 256],
                                kb[:, q2 * 256:(q2 + 1) * 256])
                    d_chunk(0, kc, scs0, ms0)
                softmax_head(0, scs0, ms0)
                exps(0)

            psO = jctx.enter_context(tc.tile_pool(name="psO", bufs=1, space="PSUM"))
            for step in range(1, 9):
                have_d = step < 8
                jt, je = step, step - 1
                if have_d:
                    scs, ms = [], []
                    d_chunk(jt, 0, scs, ms)
                    d_chunk(jt, 1, scs, ms)
                p_e = state[je][0]
                po = psO.tile([128, D + 1], f32, tag="po", bufs=1, name="po",
                              padded_shape=[128, 512])
                pt_sbs = {}
                t_half(p_e, 0, pt_sbs)
                if have_d:
                    d_chunk(jt, 2, scs, ms)
                    d_chunk(jt, 3, scs, ms)
                    softmax_head(jt, scs, ms)
                t_half(p_e, 1, pt_sbs)
                for kt in range(NKT):
                    nc.tensor.matmul(po[:],
                                     pt_sbs[kt // 8][:, (kt % 8) * 128:(kt % 8 + 1) * 128],
                                     keynf[kt // 4][:, (kt % 4) * 257:(kt % 4) * 257 + 257],
                                     start=(kt == 0), stop=(kt == NKT - 1))
                recip = stats.tile([128, 1], f32, tag="recip", bufs=2, name="recip")
                nc.vector.reciprocal(recip[:], po[:, D:D + 1])
                if have_d:
                    exps(jt)
                osb = work.tile([128, D], f32, tag="osb", bufs=2, name="osb")
                nc.scalar.mul(osb[:], po[:, :D], recip[:])
                nc.sync.dma_start(dram["ob"][je * 128:(je + 1) * 128, :], osb[:])
                state.pop(je)


def _host_prep(query, key, Wq, bq, Wk, bk):
    """Per-core input maps: double-folded transposed query, cosine tables,
    transposed key, bf16 [key|1] value matrix."""
    query = np.ascontiguousarray(query, dtype=np.float32)
    key = np.ascontiguousarray(key, dtype=np.float32)

    wqt = np.ascontiguousarray(Wq.T, dtype=np.float32) * np.float32(SCALE)
    wkn = np.ascontiguousarray(Wk, dtype=np.float32)
    bqs = np.ascontiguousarray((np.asarray(bq, dtype=np.float32) * SCALE).reshape(D, 1))
    ident = np.eye(128, dtype=ml_dtypes.bfloat16)
    ident32 = np.eye(128, dtype=np.float32)

    in_maps = []
    for b in range(B):
        x = query[b]
        y = np.empty((1025, D), dtype=np.float32)
        y[0] = x[0]
        y[1:1024] = x[1:1024] + x[2047:1024:-1]
        y[1024] = x[1024]
        yEO = np.empty((1025, D), dtype=np.float32)
        yEO[0] = y[0] + y[1024]
        yEO[1:512] = y[1:512] + y[1023:512:-1]
        yEO[512] = y[512]
        yEO[513] = y[0] - y[1024]
        yEO[514:1025] = y[1:512] - y[1023:512:-1]
        kb = key[b]
        in_maps.append({
            "yt": np.ascontiguousarray(yEO.T),
            "wqt": wqt,
            "bqs": bqs,
            "wk": wkn,
            "keyt": np.ascontiguousarray(kb.T),
            "ident": ident,
            "ident32": ident32,
        })
    return in_maps


def _host_row1024(query, key, Wq, bq, Wk, bk):
    """Row j=1024 of the output for every batch (one row of attention each),
    plus caches nothing: O(N*D) per batch."""
    rows = np.empty((B, D), dtype=np.float32)
    sgn = ((-1.0) ** np.arange(1025)).astype(np.float64)
    for b in range(B):
        x = query[b].astype(np.float64)
        y = np.empty((1025, D), dtype=np.float64)
        y[0] = x[0]
        y[1:1024] = x[1:1024] + x[2047:1024:-1]
        y[1024] = x[1024]
        q = sgn @ y                                   # Re(FFT) row 1024
        q = q @ np.asarray(Wq, np.float64).T + np.asarray(bq, np.float64)
        kv = key[b].astype(np.float64)
        s = (kv @ (np.asarray(Wk, np.float64).T @ q)
             + np.asarray(bk, np.float64) @ q) * SCALE
        s -= s.max()
        p = np.exp(s)
        rows[b] = (p @ kv / p.sum()).astype(np.float32)
    return rows


def kernel(query, key, Wq, bq, Wk, bk, _trace=False, _trace_kwargs=None):
    if "nc" not in _compiled:
        _compiled["nc"] = _build_module()
    nc = _compiled["nc"]

    in_maps = _host_prep(query, key, Wq, bq, Wk, bk)
    kw = {}
    if _trace:
        kw["trace"] = True
        if _trace_kwargs:
            kw.update(_trace_kwargs)
    res = run_bass_kernel_spmd(nc, in_maps, core_ids=list(range(B)), **kw)
    _compiled["last_results"] = res

    row1024 = _host_row1024(query, key, Wq, bq, Wk, bk)
    out = np.empty((B, NSEQ, D), dtype=np.float32)
    for b in range(B):
        ob = res.results[b]["ob"]
        out[b, 0:NJ:2] = ob[:512]
        out[b, 1:NJ:2] = ob[512:]
        out[b, 1024] = row1024[b]
        out[b, 1025:] = out[b, 1023:0:-1]
    return out
